# revision 29
# baseline (speedup 1.0000x reference)
"""Trainium2 Bass kernel for nn_BoundarySuppressionWithSmoothing.

Full inputs: x [8,1,512,1024] f32, prediction [8,1,512,1024] int32.
Sharding: pure data parallel, image i -> core i.

Per-core algorithm (image I [512,1024], layout A: 4 row-chunks of [128,1024]):
  - boundary detection via exp-encoded morphology on PE + ACT (exp/ln-free
    product compare), masks m3..m0 via a mask-carried dilation chain
  - 4 iterations of masked 3x3 box average with replication padding
  - separable dilated 7x7 Gaussian (dilation 6) via PE banded matmuls

Host I/O is compressed for the axon tunnel: x ships as fp16, prediction as
int8, y returns as fp16 (converted back to f32 host-side). The value path
runs in fp16 on-device (DVE 2-byte fast modes); the mask/count path stays
bf16 (exact small ints). The compiled executable, weight pack, and output
scratch buffer are cached device-resident so warm calls only move x/pred
in and y out.
"""
import math
import sys
from contextlib import ExitStack

import numpy as np

sys.path.insert(0, '/opt/trn_rl_repo')

import concourse.bass as bass  # noqa: E402
import concourse.bacc as bacc  # noqa: E402
import concourse.tile as tile  # noqa: E402
from concourse import mybir  # noqa: E402

P = 128
W = 1024
H = 512
CH = 4          # row chunks
B = 8           # batch == cores
ALPHA = 4.6     # morphology exp-encoding scale
PTHR = float(np.exp(4.2))   # product threshold for boundary test
DT = mybir.dt
AF = mybir.ActivationFunctionType
OP = mybir.AluOpType


# ---------------------------------------------------------------- weights ---
def _gauss1d():
    size, sigma = 7, 1.0
    u = np.exp(-((np.arange(size) - 3.0) ** 2) / (2 * sigma ** 2))
    # 2D reference kernel is outer(u,u)/sum => separable 1D = u/sum(u)
    return (u / u.sum()).astype(np.float64)


def build_host_consts():
    """All constant weight matrices, as one dict of fp32 arrays [128,x]."""
    c = {}
    tri = np.zeros((P, P), np.float32)
    for k in range(P):
        for d in (-1, 0, 1):
            if 0 <= k + d < P:
                tri[k, k + d] = 1.0   # lhsT[k,m]: out m from in k, |k-m|<=1
    c['T_mid'] = tri
    t_top = tri.copy(); t_top[0, 0] = 2.0
    c['T_top'] = t_top
    t_bot = tri.copy(); t_bot[P - 1, P - 1] = 2.0
    c['T_bot'] = t_bot
    t_up = np.zeros((P, P), np.float32); t_up[P - 1, 0] = 1.0
    c['T_up'] = t_up
    t_dn = np.zeros((P, P), np.float32); t_dn[0, P - 1] = 1.0
    c['T_dn'] = t_dn
    c['I'] = np.eye(P, dtype=np.float32)
    bvec = np.zeros((P, P), np.float32)
    bvec[:, 0] = -4.0; bvec[0, 0] = -3.0      # bv_top
    bvec[:, 1] = -4.0; bvec[P - 1, 1] = -3.0  # bv_bot
    c['BVEC'] = bvec

    g = _gauss1d()
    for j in range(7):
        c[f'G{j}'] = (np.eye(P) * g[j]).astype(np.float16).astype(np.float32)
    # vertical gaussian: Wv[R,S] = sum_j g[j] [clamp(R+6(j-3),0,H-1)==S]
    Wv = np.zeros((H, H), np.float64)
    for R in range(H):
        for j in range(7):
            S = min(max(R + 6 * (j - 3), 0), H - 1)
            Wv[R, S] += g[j]
    for c_dst in range(CH):
        for c_src in range(CH):
            if abs(c_dst - c_src) > 1:
                continue
            blk = Wv[c_dst * P:(c_dst + 1) * P, c_src * P:(c_src + 1) * P]
            if not blk.any():
                continue
            # lhsT[k,m] = Wv[dst=128c+m, src=128c'+k]
            c[f'B_{c_dst}_{c_src}'] = (
                np.ascontiguousarray(blk.T).astype(np.float16).astype(np.float32))
    return c


# ----------------------------------------------------------------- kernel ---
def build_kernel(ctx: ExitStack, tc: "tile.TileContext", outs, ins, reps=1):
    for _ in range(reps):
        _emit_once(ctx, tc, outs, ins)


def _emit_once(ctx: ExitStack, tc: "tile.TileContext", outs, ins):
    nc = tc.nc
    y = outs[0]                       # [512,1024] fp16 DRAM
    x, pred, wpack = ins              # x fp16, pred int8, wpack f32 DRAM

    consts = build_host_consts()
    wnames = sorted(consts.keys())

    if not hasattr(tc, '_bs_pools'):
        tc._bs_pools = (
            ctx.enter_context(tc.tile_pool(name="sb", bufs=1)),
            ctx.enter_context(tc.tile_pool(name="sbR", bufs=2)),
            ctx.enter_context(tc.tile_pool(name="wp", bufs=1)),
            ctx.enter_context(tc.tile_pool(name="psB", bufs=2, space="PSUM")),
            ctx.enter_context(tc.tile_pool(name="psY", bufs=2, space="PSUM")))
    sb, sbR, wpool, psB, psY = tc._bs_pools

    # ---- persistent image buffers ----
    lab = [sb.tile([P, W], DT.float32, name=f"lab{c}", tag=f"lab{c}") for c in range(CH)]
    OA = [sb.tile([P, W], DT.float16, name=f"OA{c}", tag=f"OA{c}") for c in range(CH)]
    OB = [sb.tile([P, W], DT.float16, name=f"OB{c}", tag=f"OB{c}") for c in range(CH)]
    for c in range(CH):
        nc.sync.dma_start(OA[c][:], x[c * P:(c + 1) * P, :])
        pv = OB[c][:].bitcast(DT.int8)[:, 0:W]
        nc.sync.dma_start(pv, pred[c * P:(c + 1) * P, :])
    for c in range(CH):
        pv = OB[c][:].bitcast(DT.int8)[:, 0:W]
        nc.vector.tensor_copy(lab[c][:], pv)

    # ---- load + prepare weights ----
    wstage = sb.tile([P, len(wnames) * P], DT.float32, tag="wstage")
    nc.sync.dma_start(wstage[:], wpack[:, :len(wnames) * P])
    wt = {}
    BF16_W = {'T_mid', 'T_top', 'T_bot', 'T_up', 'T_dn', 'I'}
    for i, name in enumerate(wnames):
        if name == 'BVEC':
            continue
        src = wstage[:, i * P:(i + 1) * P]
        dt_w = DT.bfloat16 if name in BF16_W else DT.float16
        t = wpool.tile([P, P], dt_w, name=f"w_{name}", tag=f"w_{name}")
        nc.vector.tensor_copy(t[:], src)
        wt[name] = t
    # fp16 variants of vertical matrices for the value path
    for name in ('T_mid', 'T_top', 'T_bot', 'T_up', 'T_dn'):
        t = wpool.tile([P, P], DT.float16, name=f"wr_{name}", tag=f"wr_{name}")
        i = wnames.index(name)
        nc.vector.tensor_copy(t[:], wstage[:, i * P:(i + 1) * P])
        wt['R' + name[1:]] = t

    def TRv(c):
        return wt['T_top'] if c == 0 else (wt['T_bot'] if c == CH - 1 else wt['T_mid'])

    def Rv(c):
        return wt['R_top'] if c == 0 else (wt['R_bot'] if c == CH - 1 else wt['R_mid'])

    # ---- const bias vectors ----
    def make_const(val, tag):
        t = sb.tile([P, 1], DT.float32, tag=tag)
        nc.vector.memset(t[:], val)
        return t

    b_enc_max = make_const(-9.0 * ALPHA, "b_enc_max")
    b_enc_min = make_const(+9.0 * ALPHA, "b_enc_min")
    bv_mid = make_const(-4.0, "bv_mid")
    ib = wnames.index('BVEC')
    bv_top = sb.tile([P, 1], DT.float32, name="bv_top", tag="bv_top")
    nc.vector.tensor_copy(bv_top[:], wstage[:, ib * P:ib * P + 1])
    bv_bot = sb.tile([P, 1], DT.float32, name="bv_bot", tag="bv_bot")
    nc.vector.tensor_copy(bv_bot[:], wstage[:, ib * P + 1:ib * P + 2])
    one_c = make_const(1.0, "one_c")

    def bv(c):
        return bv_top if c == 0 else (bv_bot if c == CH - 1 else bv_mid)

    GW = W + 2

    def gtile(tag, dtype, guard_val, pool=sb):
        ts = [pool.tile([P, GW], dtype, name=f"{tag}{c}", tag=f"{tag}{c}") for c in range(CH)]
        for c in range(CH):
            for ap in (ts[c][:, 0:1], ts[c][:, GW - 1:GW]):
                nc.vector.memset(ap, guard_val)
        return ts

    Emax = gtile("Emax", DT.bfloat16, 0.0)
    Emin = gtile("Emin", DT.bfloat16, 0.0)
    m = [gtile(f"m{i}_", DT.bfloat16, 1.0) for i in range(4)]
    xm = gtile("xm", DT.float16, 0.0)
    HN = [sb.tile([P, W], DT.bfloat16, name=f"HN{c}", tag=f"HMa{c}") for c in range(CH)]
    HMa = [sb.tile([P, W], DT.bfloat16, name=f"HMa{c}", tag=f"HMa{c}") for c in range(CH)]
    hlr = [sb.tile([P, W], DT.float16, name=f"hlr{c}", tag=f"hlr{c}") for c in range(CH)]

    def data(t):
        return t[:, 1:W + 1]

    def shl(t):
        return t[:, 0:W]

    def shr(t):
        return t[:, 2:W + 2]

    def pool_copy_predicated(out, mask, dat):
        eng = nc.gpsimd
        eng.add_instruction(mybir.InstCopyPredicated(
            name=f"I-{eng.bass.next_id()}",
            ins=[eng.lower_ap(mask), eng.lower_ap(dat)],
            outs=[eng.lower_ap(out)]))

    def mm_group(pt, pairs):
        # split into N=512 sub-matmuls (PSUM bank limit); weight-major order
        # so consecutive matmuls share the stationary operand (fewer LDW).
        n = pt.shape[1]
        halves = list(range(0, n, 512))
        for i, (lhsT, rhs) in enumerate(pairs):
            for h0 in halves:
                nc.tensor.matmul(pt[:, h0:h0 + 512], lhsT,
                                 rhs[:, h0:h0 + 512], start=(i == 0),
                                 stop=(i == len(pairs) - 1))

    # ================= Phase M: encode + boundary masks ===================
    for c in range(CH):
        nc.scalar.activation(data(Emax[c]), lab[c][:], AF.Exp,
                             bias=b_enc_max[:], scale=ALPHA)
        nc.scalar.activation(data(Emin[c]), lab[c][:], AF.Exp,
                             bias=b_enc_min[:], scale=-ALPHA)
    # horizontal presums (DVE, bf16 fast mode)
    SX = [sb.tile([P, W], DT.bfloat16, name=f"SX{c}", tag=f"SX{c}") for c in range(CH)]
    for c in range(CH):
        nc.vector.tensor_tensor(HN[c][:], shl(Emin[c]), shr(Emin[c]), op=OP.add)
        nc.vector.tensor_tensor(HN[c][:], HN[c][:], data(Emin[c]), op=OP.add)
        nc.vector.tensor_tensor(SX[c][:], shl(Emax[c]), shr(Emax[c]), op=OP.add)
    for c in range(CH):
        p1 = psB.tile([P, W], DT.float32, name="pS1", tag="psb")
        pairs = [(wt['T_mid'][:], data(Emax[c])),
                 (wt['I'][:], SX[c][:])]
        if c > 0:
            pairs.append((wt['T_up'][:], data(Emax[c - 1])))
        if c < CH - 1:
            pairs.append((wt['T_dn'][:], data(Emax[c + 1])))
        mm_group(p1[:], pairs)
        sc1 = sbR.tile([P, W], DT.bfloat16, name="sc1", tag="nb")
        nc.scalar.copy(sc1[:], p1[:])

        p2 = psB.tile([P, W], DT.float32, name="pS2", tag="psb")
        pairs = [(wt['T_mid'][:], HN[c][:])]
        if c > 0:
            pairs.append((wt['T_up'][:], HN[c - 1][:]))
        if c < CH - 1:
            pairs.append((wt['T_dn'][:], HN[c + 1][:]))
        mm_group(p2[:], pairs)
        pb = sbR.tile([P, W], DT.bfloat16, name="pb", tag="zt")
        nc.vector.tensor_tensor(pb[:], sc1[:], p2[:], op=OP.mult)
        nc.vector.tensor_scalar(data(m[3][c]), pb[:], PTHR, None, op0=OP.is_lt)

    # ================= Chain: m3 -> m2 -> m1 -> m0 ========================
    # (erosion semantics need guard cols = 1.0 while a mask is a chain input;
    # after its last chain use, guards are replicated for the U loop's
    # replication-padded box sums)
    for k in range(3):
        mp, mn = m[3 - k], m[2 - k]
        for c in range(CH):
            sm = sbR.tile([P, W], DT.bfloat16, name="sm", tag="sm")
            nc.vector.tensor_tensor(sm[:], shl(mp[c]), shr(mp[c]), op=OP.add)
            ps = psB.tile([P, W], DT.float32, name="pCh", tag="psb")
            pairs = [(wt['T_mid'][:], data(mp[c])),
                     (wt['I'][:], sm[:])]
            if c > 0:
                pairs.append((wt['T_up'][:], data(mp[c - 1])))
            if c < CH - 1:
                pairs.append((wt['T_dn'][:], data(mp[c + 1])))
            mm_group(ps[:], pairs)
            nc.scalar.activation(data(mn[c]), ps[:], AF.Relu, bias=bv(c)[:],
                                 scale=1.0)
        for c in range(CH):  # mp fully consumed: replicate guards for U loop
            nc.vector.tensor_copy(mp[c][:, 0:1], mp[c][:, 1:2])
            nc.vector.tensor_copy(mp[c][:, GW - 1:GW], mp[c][:, W:W + 1])
    for c in range(CH):
        nc.vector.tensor_copy(m[0][c][:, 0:1], m[0][c][:, 1:2])
        nc.vector.tensor_copy(m[0][c][:, GW - 1:GW], m[0][c][:, W:W + 1])

    # ================= U loop =============================================
    GA = 18
    gs = [sb.tile([P, W + 2 * GA], DT.float16, name=f"gs{c}", tag=f"lab{c}")
          for c in range(CH)]
    hg = [sb.tile([P, W], DT.float16, name=f"Emin{c}", tag=f"Emin{c}") for c in range(CH)]
    yo = [sb.tile([P, W], DT.float16, name=f"Emax{c}", tag=f"Emax{c}") for c in range(CH)]

    def emit_gauss_h(c, src):
        # horizontal dilated gaussian for chunk c, emitted as soon as the
        # final U-iteration output for c lands (overlaps the U-loop tail)
        nc.vector.tensor_copy(gs[c][:, GA:GA + W], src[:])
        nc.vector.tensor_copy(gs[c][:, 0:GA], src[:, 0:1].to_broadcast((P, GA)))
        nc.vector.tensor_copy(gs[c][:, GA + W:],
                              src[:, W - 1:W].to_broadcast((P, GA)))
        for h in range(2):
            ph = psY.tile([P, 512], DT.float32, name="pH", tag="psy")
            for j in range(7):
                off = GA + 6 * (j - 3) + h * 512
                nc.tensor.matmul(ph[:], wt[f'G{j}'][:], gs[c][:, off:off + 512],
                                 start=(j == 0), stop=(j == 6))
            nc.scalar.copy(hg[c][:, h * 512:(h + 1) * 512], ph[:])

    cur, nxt = OA, OB
    for it in range(4):
        mi = m[it]
        for c in range(CH):
            # all on DVE: 2-byte operands hit the 2x/4x fast modes
            nc.vector.tensor_tensor(xm[c][:, 1:W + 1], cur[c][:], data(mi[c]),
                                    op=OP.mult)
            nc.vector.tensor_tensor(hlr[c][:], shl(xm[c]), shr(xm[c]), op=OP.add)
            nc.vector.tensor_tensor(hlr[c][:, 0:1], hlr[c][:, 0:1],
                                    xm[c][:, 1:2], op=OP.add)
            nc.vector.tensor_tensor(hlr[c][:, W - 1:W], hlr[c][:, W - 1:W],
                                    xm[c][:, W:W + 1], op=OP.add)
            nc.vector.tensor_tensor(hlr[c][:], hlr[c][:], xm[c][:, 1:W + 1],
                                    op=OP.add)
            nc.vector.tensor_tensor(HMa[c][:], shl(mi[c]), shr(mi[c]), op=OP.add)
            nc.vector.tensor_tensor(HMa[c][:], HMa[c][:], data(mi[c]), op=OP.add)
        MkL, nbL = [], []
        for c in range(CH):
            pn = psB.tile([P, W], DT.float32, name="pN", tag="psb")
            pairs = [(TRv(c)[:], HMa[c][:])]
            if c > 0:
                pairs.append((wt['T_up'][:], HMa[c - 1][:]))
            if c < CH - 1:
                pairs.append((wt['T_dn'][:], HMa[c + 1][:]))
            mm_group(pn[:], pairs)
            zt = sbR.tile([P, W], DT.bfloat16, name="zt", tag="zt")
            nc.scalar.activation(zt[:], pn[:], AF.Relu, bias=one_c[:],
                                 scale=-1.0)
            Mk = sbR.tile([P, W], DT.int16, name="Mk", tag=f"Mk{c % 2}")
            nc.vector.tensor_tensor(Mk[:], data(mi[c]), zt[:], op=OP.add)
            MkL.append(Mk)
            nb = sbR.tile([P, W], DT.float16, name="nb", tag=f"nsb{c % 2}")
            with nc.allow_low_precision(reason="1/n of exact small counts"):
                nc.vector.reciprocal(nb[:], pn[:])
            nbL.append(nb)
        for c in range(CH):
            pyt = psY.tile([P, W], DT.float32, name="pY", tag="psy")
            pairs = [(Rv(c)[:], hlr[c][:])]
            if c > 0:
                pairs.append((wt['R_up'][:], hlr[c - 1][:]))
            if c < CH - 1:
                pairs.append((wt['R_dn'][:], hlr[c + 1][:]))
            mm_group(pyt[:], pairs)
            # avg = Y * (1/n); n==0 -> inf/nan, overwritten below. Neither
            # Pool nor DVE has a divide op, so: ACT drains Y to SBUF, DVE
            # takes 1/n from PSUM, DVE multiplies (2-byte SBUF fast mode).
            ysb = sbR.tile([P, W], DT.float16, name="ysb", tag="ysb")
            nc.scalar.copy(ysb[:], pyt[:])
            nc.vector.tensor_tensor(nxt[c][:], ysb[:], nbL[c][:], op=OP.mult)
            nc.vector.copy_predicated(nxt[c][:], MkL[c][:], cur[c][:])
        cur, nxt = nxt, cur

    # ================= Gaussian ==========================================
    for c in range(CH):
        emit_gauss_h(c, cur[c])
    for c in range(CH):
        pv = psY.tile([P, W], DT.float32, name="pV", tag="psy")
        srcs = [cc for cc in range(CH) if f'B_{c}_{cc}' in wt]
        mm_group(pv[:], [(wt[f'B_{c}_{cc}'][:], hg[cc][:]) for cc in srcs])
        nc.scalar.copy(yo[c][:], pv[:])
    for c in range(CH):
        nc.sync.dma_start(y[c * P:(c + 1) * P, :], yo[c][:])


# ------------------------------------------------------------ host driver ---
_CACHE = {}


def _build_program(reps=1):
    key = ('nc', reps)
    if key in _CACHE:
        return _CACHE[key], _CACHE['wpack']
    consts = build_host_consts()
    wnames = sorted(consts.keys())
    wpack = np.zeros((P, len(wnames) * P), np.float32)
    for i, n in enumerate(wnames):
        wpack[:, i * P:(i + 1) * P] = consts[n]

    nc = bacc.Bacc("TRN2", target_bir_lowering=False, debug=False,
                   num_devices=B)
    x_d = nc.dram_tensor("x", [H, W], DT.float16, kind="ExternalInput").ap()
    p_d = nc.dram_tensor("prediction", [H, W], DT.int8,
                         kind="ExternalInput").ap()
    w_d = nc.dram_tensor("wpack", list(wpack.shape), DT.float32,
                         kind="ExternalInput").ap()
    y_d = nc.dram_tensor("y", [H, W], DT.float16, kind="ExternalOutput").ap()
    with tile.TileContext(nc) as tc:
        with ExitStack() as ctx:
            build_kernel(ctx, tc, [y_d], [x_d, p_d, w_d], reps=reps)
    nc.compile()
    _CACHE[('nc', reps)] = nc
    _CACHE['wpack'] = wpack
    return nc, wpack


def _get_exec(reps=1):
    """Compile (once) the 8-core sharded executable; stage constants."""
    key = ('exec', reps)
    if key in _CACHE:
        return _CACHE[key]
    import jax
    from jax.sharding import Mesh, PartitionSpec, NamedSharding
    from jax.experimental.shard_map import shard_map
    from concourse import bass2jax

    bass2jax.install_neuronx_cc_hook()
    nc, wpack = _build_program(reps)

    partition_name = (nc.partition_id_tensor.name
                      if nc.partition_id_tensor else None)
    in_names, out_names, out_avals = [], [], []
    for alloc in nc.m.functions[0].allocations:
        if not isinstance(alloc, mybir.MemoryLocationSet):
            continue
        name = alloc.memorylocations[0].name
        if alloc.kind == "ExternalInput":
            if name != partition_name:
                in_names.append(name)
        elif alloc.kind == "ExternalOutput":
            out_names.append(name)
            out_avals.append(jax.core.ShapedArray(
                tuple(alloc.tensor_shape), mybir.dt.np(alloc.dtype)))
    n_params = len(in_names)
    n_outs = len(out_names)
    all_names = list(in_names) + list(out_names)
    if partition_name is not None:
        all_names.append(partition_name)

    def _body(*args):
        operands = list(args)
        if partition_name is not None:
            operands.append(bass2jax.partition_id_tensor())
        outs = bass2jax._bass_exec_p.bind(
            *operands, out_avals=tuple(out_avals), in_names=tuple(all_names),
            out_names=tuple(out_names), lowering_input_output_aliases=(),
            sim_require_finite=True, sim_require_nnan=True, nc=nc)
        return tuple(outs)

    devices = jax.devices()[:B]
    mesh = Mesh(np.asarray(devices), ("core",))
    shard = NamedSharding(mesh, PartitionSpec("core"))

    # global-shape avals for AOT lowering (axis 0 concatenated over cores)
    assert in_names == ['x', 'prediction', 'wpack'], in_names
    arg_shapes = [
        jax.ShapeDtypeStruct((B * H, W), np.float16, sharding=shard),
        jax.ShapeDtypeStruct((B * H, W), np.int8, sharding=shard),
        jax.ShapeDtypeStruct((B * wpack.shape[0], wpack.shape[1]), np.float32,
                             sharding=shard),
        jax.ShapeDtypeStruct((B * H, W), np.float16, sharding=shard),
    ]

    def compile_fn():
        jf = jax.jit(shard_map(
            _body, mesh=mesh,
            in_specs=(PartitionSpec("core"),) * (n_params + n_outs),
            out_specs=(PartitionSpec("core"),) * n_outs,
            check_rep=False), keep_unused=True)
        return jf.lower(*arg_shapes).compile()

    try:
        compiled = bass2jax.fast_dispatch_compile(compile_fn)
    except Exception:
        compiled = compile_fn()

    wd = jax.device_put(np.concatenate([wpack] * B, axis=0), shard)
    zd = jax.device_put(np.zeros((B * H, W), np.float16), shard)
    jax.block_until_ready((wd, zd))

    st = {'compiled': compiled, 'shard': shard, 'wd': wd, 'zd': zd,
          'nc': nc, 'wpack': wpack}
    _CACHE[('exec', reps)] = st
    return st


def _stage_inputs(x, prediction):
    """Host-compress + device_put with the executable's sharding."""
    import jax
    st = _get_exec()
    xs = np.ascontiguousarray(x.reshape(B * H, W)).astype(np.float16)
    ps = prediction.reshape(B * H, W).astype(np.int8)
    xd = jax.device_put(xs, st['shard'])
    pd = jax.device_put(np.ascontiguousarray(ps), st['shard'])
    return xd, pd


def kernel(x: np.ndarray, prediction: np.ndarray) -> np.ndarray:
    st = _get_exec()
    xd, pd = _stage_inputs(x, prediction)
    out = st['compiled'](xd, pd, st['wd'], st['zd'])
    y = np.asarray(out[0]).astype(np.float32).reshape(B, 1, H, W)
    return y


if __name__ == "__main__":
    xs = np.random.randn(B, 1, H, W).astype(np.float32)
    ps = np.random.randint(0, 19, size=(B, 1, H, W)).astype(np.int32)
    print(kernel(xs, ps).shape)


# revision 31
# speedup vs baseline: 1.0064x; 1.0064x over previous
"""Trainium2 Bass kernel for nn_BoundarySuppressionWithSmoothing.

Full inputs: x [8,1,512,1024] f32, prediction [8,1,512,1024] int32.
Sharding: pure data parallel, image i -> core i.

Per-core algorithm (image I [512,1024], layout A: 4 row-chunks of [128,1024]):
  - boundary detection via exp-encoded morphology on PE + ACT (exp/ln-free
    product compare), masks m3..m0 via a mask-carried dilation chain
  - 4 iterations of masked 3x3 box average with replication padding
  - separable dilated 7x7 Gaussian (dilation 6) via PE banded matmuls

Host I/O is compressed for the axon tunnel: x ships as fp16, prediction as
int8, y returns as fp16 (converted back to f32 host-side). The value path
runs in fp16 on-device (DVE 2-byte fast modes); the mask/count path stays
bf16 (exact small ints). The compiled executable, weight pack, and output
scratch buffer are cached device-resident so warm calls only move x/pred
in and y out.
"""
import math
import sys
from contextlib import ExitStack

import numpy as np

sys.path.insert(0, '/opt/trn_rl_repo')

import concourse.bass as bass  # noqa: E402
import concourse.bacc as bacc  # noqa: E402
import concourse.tile as tile  # noqa: E402
from concourse import mybir  # noqa: E402

P = 128
W = 1024
H = 512
CH = 4          # row chunks
B = 8           # batch == cores
ALPHA = 4.6     # morphology exp-encoding scale
PTHR = float(np.exp(4.2))   # product threshold for boundary test
DT = mybir.dt
AF = mybir.ActivationFunctionType
OP = mybir.AluOpType


# ---------------------------------------------------------------- weights ---
def _gauss1d():
    size, sigma = 7, 1.0
    u = np.exp(-((np.arange(size) - 3.0) ** 2) / (2 * sigma ** 2))
    # 2D reference kernel is outer(u,u)/sum => separable 1D = u/sum(u)
    return (u / u.sum()).astype(np.float64)


def build_host_consts():
    """All constant weight matrices, as one dict of fp32 arrays [128,x]."""
    c = {}
    tri = np.zeros((P, P), np.float32)
    for k in range(P):
        for d in (-1, 0, 1):
            if 0 <= k + d < P:
                tri[k, k + d] = 1.0   # lhsT[k,m]: out m from in k, |k-m|<=1
    c['T_mid'] = tri
    t_top = tri.copy(); t_top[0, 0] = 2.0
    c['T_top'] = t_top
    t_bot = tri.copy(); t_bot[P - 1, P - 1] = 2.0
    c['T_bot'] = t_bot
    t_up = np.zeros((P, P), np.float32); t_up[P - 1, 0] = 1.0
    c['T_up'] = t_up
    t_dn = np.zeros((P, P), np.float32); t_dn[0, P - 1] = 1.0
    c['T_dn'] = t_dn
    c['I'] = np.eye(P, dtype=np.float32)
    bvec = np.zeros((P, P), np.float32)
    bvec[:, 0] = -4.0; bvec[0, 0] = -3.0      # bv_top
    bvec[:, 1] = -4.0; bvec[P - 1, 1] = -3.0  # bv_bot
    c['BVEC'] = bvec

    g = _gauss1d()
    for j in range(7):
        c[f'G{j}'] = (np.eye(P) * g[j]).astype(np.float16).astype(np.float32)
    # vertical gaussian: Wv[R,S] = sum_j g[j] [clamp(R+6(j-3),0,H-1)==S]
    Wv = np.zeros((H, H), np.float64)
    for R in range(H):
        for j in range(7):
            S = min(max(R + 6 * (j - 3), 0), H - 1)
            Wv[R, S] += g[j]
    for c_dst in range(CH):
        for c_src in range(CH):
            if abs(c_dst - c_src) > 1:
                continue
            blk = Wv[c_dst * P:(c_dst + 1) * P, c_src * P:(c_src + 1) * P]
            if not blk.any():
                continue
            # lhsT[k,m] = Wv[dst=128c+m, src=128c'+k]
            c[f'B_{c_dst}_{c_src}'] = (
                np.ascontiguousarray(blk.T).astype(np.float16).astype(np.float32))
    return c


# ----------------------------------------------------------------- kernel ---
def build_kernel(ctx: ExitStack, tc: "tile.TileContext", outs, ins, reps=1):
    for _ in range(reps):
        _emit_once(ctx, tc, outs, ins)


def _emit_once(ctx: ExitStack, tc: "tile.TileContext", outs, ins):
    nc = tc.nc
    y = outs[0]                       # [512,1024] fp16 DRAM
    x, pred, wpack = ins              # x fp16, pred int8, wpack f32 DRAM

    consts = build_host_consts()
    wnames = sorted(consts.keys())

    if not hasattr(tc, '_bs_pools'):
        tc._bs_pools = (
            ctx.enter_context(tc.tile_pool(name="sb", bufs=1)),
            ctx.enter_context(tc.tile_pool(name="sbR", bufs=2)),
            ctx.enter_context(tc.tile_pool(name="wp", bufs=1)),
            ctx.enter_context(tc.tile_pool(name="psB", bufs=2, space="PSUM")),
            ctx.enter_context(tc.tile_pool(name="psY", bufs=2, space="PSUM")))
    sb, sbR, wpool, psB, psY = tc._bs_pools

    # ---- persistent image buffers ----
    lab = [sb.tile([P, W], DT.float32, name=f"lab{c}", tag=f"lab{c}") for c in range(CH)]
    OA = [sb.tile([P, W], DT.float16, name=f"OA{c}", tag=f"OA{c}") for c in range(CH)]
    OB = [sb.tile([P, W], DT.float16, name=f"OB{c}", tag=f"OB{c}") for c in range(CH)]
    for c in range(CH):
        nc.sync.dma_start(OA[c][:], x[c * P:(c + 1) * P, :])
        pv = OB[c][:].bitcast(DT.int8)[:, 0:W]
        nc.sync.dma_start(pv, pred[c * P:(c + 1) * P, :])
    for c in range(CH):
        pv = OB[c][:].bitcast(DT.int8)[:, 0:W]
        nc.vector.tensor_copy(lab[c][:], pv)

    # ---- load + prepare weights ----
    wstage = sb.tile([P, len(wnames) * P], DT.float32, tag="wstage")
    nc.sync.dma_start(wstage[:], wpack[:, :len(wnames) * P])
    wt = {}
    BF16_W = {'T_mid', 'T_top', 'T_bot', 'T_up', 'T_dn', 'I'}
    for i, name in enumerate(wnames):
        if name == 'BVEC':
            continue
        src = wstage[:, i * P:(i + 1) * P]
        dt_w = DT.bfloat16 if name in BF16_W else DT.float16
        t = wpool.tile([P, P], dt_w, name=f"w_{name}", tag=f"w_{name}")
        nc.vector.tensor_copy(t[:], src)
        wt[name] = t
    # fp16 variants of vertical matrices for the value path
    for name in ('T_mid', 'T_top', 'T_bot', 'T_up', 'T_dn'):
        t = wpool.tile([P, P], DT.float16, name=f"wr_{name}", tag=f"wr_{name}")
        i = wnames.index(name)
        nc.vector.tensor_copy(t[:], wstage[:, i * P:(i + 1) * P])
        wt['R' + name[1:]] = t

    def TRv(c):
        return wt['T_top'] if c == 0 else (wt['T_bot'] if c == CH - 1 else wt['T_mid'])

    def Rv(c):
        return wt['R_top'] if c == 0 else (wt['R_bot'] if c == CH - 1 else wt['R_mid'])

    # ---- const bias vectors ----
    def make_const(val, tag):
        t = sb.tile([P, 1], DT.float32, tag=tag)
        nc.vector.memset(t[:], val)
        return t

    b_enc_max = make_const(-9.0 * ALPHA, "b_enc_max")
    b_enc_min = make_const(+9.0 * ALPHA, "b_enc_min")
    bv_mid = make_const(-4.0, "bv_mid")
    ib = wnames.index('BVEC')
    bv_top = sb.tile([P, 1], DT.float32, name="bv_top", tag="bv_top")
    nc.vector.tensor_copy(bv_top[:], wstage[:, ib * P:ib * P + 1])
    bv_bot = sb.tile([P, 1], DT.float32, name="bv_bot", tag="bv_bot")
    nc.vector.tensor_copy(bv_bot[:], wstage[:, ib * P + 1:ib * P + 2])
    one_c = make_const(1.0, "one_c")

    def bv(c):
        return bv_top if c == 0 else (bv_bot if c == CH - 1 else bv_mid)

    GW = W + 2

    def gtile(tag, dtype, guard_val, pool=sb):
        ts = [pool.tile([P, GW], dtype, name=f"{tag}{c}", tag=f"{tag}{c}") for c in range(CH)]
        for c in range(CH):
            for ap in (ts[c][:, 0:1], ts[c][:, GW - 1:GW]):
                nc.vector.memset(ap, guard_val)
        return ts

    Emax = gtile("Emax", DT.bfloat16, 0.0)
    Emin = gtile("Emin", DT.bfloat16, 0.0)
    m = [gtile(f"m{i}_", DT.bfloat16, 1.0) for i in range(4)]
    xm = gtile("xm", DT.float16, 0.0)
    HN = [sb.tile([P, W], DT.bfloat16, name=f"HN{c}", tag=f"HMa{c}") for c in range(CH)]
    HMa = [sb.tile([P, W], DT.bfloat16, name=f"HMa{c}", tag=f"HMa{c}") for c in range(CH)]
    hlr = [sb.tile([P, W], DT.float16, name=f"hlr{c}", tag=f"hlr{c}") for c in range(CH)]

    def data(t):
        return t[:, 1:W + 1]

    def shl(t):
        return t[:, 0:W]

    def shr(t):
        return t[:, 2:W + 2]

    def pool_copy_predicated(out, mask, dat):
        eng = nc.gpsimd
        eng.add_instruction(mybir.InstCopyPredicated(
            name=f"I-{eng.bass.next_id()}",
            ins=[eng.lower_ap(mask), eng.lower_ap(dat)],
            outs=[eng.lower_ap(out)]))

    def mm_group(pt, pairs):
        # split into N=512 sub-matmuls (PSUM bank limit); weight-major order
        # so consecutive matmuls share the stationary operand (fewer LDW).
        n = pt.shape[1]
        halves = list(range(0, n, 512))
        for i, (lhsT, rhs) in enumerate(pairs):
            for h0 in halves:
                nc.tensor.matmul(pt[:, h0:h0 + 512], lhsT,
                                 rhs[:, h0:h0 + 512], start=(i == 0),
                                 stop=(i == len(pairs) - 1))

    # ================= Phase M: encode + boundary masks ===================
    for c in range(CH):
        nc.scalar.activation(data(Emax[c]), lab[c][:], AF.Exp,
                             bias=b_enc_max[:], scale=ALPHA)
        nc.scalar.activation(data(Emin[c]), lab[c][:], AF.Exp,
                             bias=b_enc_min[:], scale=-ALPHA)
    # horizontal presums (DVE, bf16 fast mode)
    SX = [sb.tile([P, W], DT.bfloat16, name=f"SX{c}", tag=f"SX{c}") for c in range(CH)]
    for c in range(CH):
        nc.vector.tensor_tensor(HN[c][:], shl(Emin[c]), shr(Emin[c]), op=OP.add)
        nc.vector.tensor_tensor(HN[c][:], HN[c][:], data(Emin[c]), op=OP.add)
        nc.vector.tensor_tensor(SX[c][:], shl(Emax[c]), shr(Emax[c]), op=OP.add)
    for c in range(CH):
        p1 = psB.tile([P, W], DT.float32, name="pS1", tag="psb")
        pairs = [(wt['T_mid'][:], data(Emax[c])),
                 (wt['I'][:], SX[c][:])]
        if c > 0:
            pairs.append((wt['T_up'][:], data(Emax[c - 1])))
        if c < CH - 1:
            pairs.append((wt['T_dn'][:], data(Emax[c + 1])))
        mm_group(p1[:], pairs)
        sc1 = sbR.tile([P, W], DT.bfloat16, name="sc1", tag="nb")
        nc.scalar.copy(sc1[:], p1[:])

        p2 = psB.tile([P, W], DT.float32, name="pS2", tag="psb")
        pairs = [(wt['T_mid'][:], HN[c][:])]
        if c > 0:
            pairs.append((wt['T_up'][:], HN[c - 1][:]))
        if c < CH - 1:
            pairs.append((wt['T_dn'][:], HN[c + 1][:]))
        mm_group(p2[:], pairs)
        pb = sbR.tile([P, W], DT.bfloat16, name="pb", tag="zt")
        nc.vector.tensor_tensor(pb[:], sc1[:], p2[:], op=OP.mult)
        nc.vector.tensor_scalar(data(m[3][c]), pb[:], PTHR, None, op0=OP.is_lt)

    # ================= Chain: m3 -> m2 -> m1 -> m0 ========================
    # (erosion semantics need guard cols = 1.0 while a mask is a chain input;
    # after its last chain use, guards are replicated for the U loop's
    # replication-padded box sums)
    for k in range(3):
        mp, mn = m[3 - k], m[2 - k]
        for c in range(CH):
            sm = sbR.tile([P, W], DT.bfloat16, name="sm", tag="sm")
            nc.vector.tensor_tensor(sm[:], shl(mp[c]), shr(mp[c]), op=OP.add)
            ps = psB.tile([P, W], DT.float32, name="pCh", tag="psb")
            pairs = [(wt['T_mid'][:], data(mp[c])),
                     (wt['I'][:], sm[:])]
            if c > 0:
                pairs.append((wt['T_up'][:], data(mp[c - 1])))
            if c < CH - 1:
                pairs.append((wt['T_dn'][:], data(mp[c + 1])))
            mm_group(ps[:], pairs)
            nc.scalar.activation(data(mn[c]), ps[:], AF.Relu, bias=bv(c)[:],
                                 scale=1.0)
        for c in range(CH):  # mp fully consumed: replicate guards for U loop
            nc.vector.tensor_copy(mp[c][:, 0:1], mp[c][:, 1:2])
            nc.vector.tensor_copy(mp[c][:, GW - 1:GW], mp[c][:, W:W + 1])
    for c in range(CH):
        nc.vector.tensor_copy(m[0][c][:, 0:1], m[0][c][:, 1:2])
        nc.vector.tensor_copy(m[0][c][:, GW - 1:GW], m[0][c][:, W:W + 1])

    # ================= U loop =============================================
    GA = 18
    gs = [sb.tile([P, W + 2 * GA], DT.float16, name=f"gs{c}", tag=f"lab{c}")
          for c in range(CH)]
    hg = [sb.tile([P, W], DT.float16, name=f"Emin{c}", tag=f"Emin{c}") for c in range(CH)]
    yo = [sb.tile([P, W], DT.float16, name=f"Emax{c}", tag=f"Emax{c}") for c in range(CH)]

    def emit_gauss_h(c, src):
        # horizontal dilated gaussian for chunk c, emitted as soon as the
        # final U-iteration output for c lands (overlaps the U-loop tail)
        nc.vector.tensor_copy(gs[c][:, GA:GA + W], src[:])
        nc.vector.tensor_copy(gs[c][:, 0:GA], src[:, 0:1].to_broadcast((P, GA)))
        nc.vector.tensor_copy(gs[c][:, GA + W:],
                              src[:, W - 1:W].to_broadcast((P, GA)))
        for h in range(2):
            ph = psY.tile([P, 512], DT.float32, name="pH", tag="psy")
            for j in range(7):
                off = GA + 6 * (j - 3) + h * 512
                nc.tensor.matmul(ph[:], wt[f'G{j}'][:], gs[c][:, off:off + 512],
                                 start=(j == 0), stop=(j == 6))
            nc.scalar.copy(hg[c][:, h * 512:(h + 1) * 512], ph[:])

    cur, nxt = OA, OB
    for it in range(4):
        mi = m[it]
        for c in range(CH):
            # all on DVE: 2-byte operands hit the 2x/4x fast modes
            nc.vector.tensor_tensor(xm[c][:, 1:W + 1], cur[c][:], data(mi[c]),
                                    op=OP.mult)
            nc.vector.tensor_tensor(hlr[c][:], shl(xm[c]), shr(xm[c]), op=OP.add)
            nc.vector.tensor_tensor(hlr[c][:, 0:1], hlr[c][:, 0:1],
                                    xm[c][:, 1:2], op=OP.add)
            nc.vector.tensor_tensor(hlr[c][:, W - 1:W], hlr[c][:, W - 1:W],
                                    xm[c][:, W:W + 1], op=OP.add)
            nc.vector.tensor_tensor(hlr[c][:], hlr[c][:], xm[c][:, 1:W + 1],
                                    op=OP.add)
            nc.vector.tensor_tensor(HMa[c][:], shl(mi[c]), shr(mi[c]), op=OP.add)
            nc.vector.tensor_tensor(HMa[c][:], HMa[c][:], data(mi[c]), op=OP.add)
        MkL, nbL = [], []
        for c in range(CH):
            pn = psB.tile([P, W], DT.float32, name="pN", tag="psb")
            pairs = [(TRv(c)[:], HMa[c][:])]
            if c > 0:
                pairs.append((wt['T_up'][:], HMa[c - 1][:]))
            if c < CH - 1:
                pairs.append((wt['T_dn'][:], HMa[c + 1][:]))
            mm_group(pn[:], pairs)
            zt = sbR.tile([P, W], DT.bfloat16, name="zt", tag="zt")
            nc.scalar.activation(zt[:], pn[:], AF.Relu, bias=one_c[:],
                                 scale=-1.0)
            Mk = sbR.tile([P, W], DT.int16, name="Mk", tag=f"Mk{c % 2}")
            nc.vector.tensor_tensor(Mk[:], data(mi[c]), zt[:], op=OP.add)
            MkL.append(Mk)
            nb = sbR.tile([P, W], DT.float16, name="nb", tag=f"nsb{c % 2}")
            with nc.allow_low_precision(reason="1/n of exact small counts"):
                nc.vector.reciprocal(nb[:], pn[:])
            nbL.append(nb)
        for c in range(CH):
            pyt = psY.tile([P, W], DT.float32, name="pY", tag="psy")
            pairs = [(Rv(c)[:], hlr[c][:])]
            if c > 0:
                pairs.append((wt['R_up'][:], hlr[c - 1][:]))
            if c < CH - 1:
                pairs.append((wt['R_dn'][:], hlr[c + 1][:]))
            mm_group(pyt[:], pairs)
            # avg = Y * (1/n); n==0 -> inf/nan, overwritten below. Neither
            # Pool nor DVE has a divide op, so: ACT drains Y to SBUF, DVE
            # takes 1/n from PSUM, DVE multiplies (2-byte SBUF fast mode).
            ysb = sbR.tile([P, W], DT.float16, name="ysb", tag="ysb")
            nc.scalar.copy(ysb[:], pyt[:])
            nc.vector.tensor_tensor(nxt[c][:], ysb[:], nbL[c][:], op=OP.mult)
            nc.vector.copy_predicated(nxt[c][:], MkL[c][:], cur[c][:])
        cur, nxt = nxt, cur

    # ================= Gaussian ==========================================
    for c in range(CH):
        emit_gauss_h(c, cur[c])
    for c in range(CH):
        pv = psY.tile([P, W], DT.float32, name="pV", tag="psy")
        srcs = [cc for cc in range(CH) if f'B_{c}_{cc}' in wt]
        mm_group(pv[:], [(wt[f'B_{c}_{cc}'][:], hg[cc][:]) for cc in srcs])
        nc.scalar.copy(yo[c][:], pv[:])
    for c in range(CH):
        nc.sync.dma_start(y[c * P:(c + 1) * P, :], yo[c][:])


# ------------------------------------------------------------ host driver ---
_CACHE = {}


def _build_program(reps=1):
    key = ('nc', reps)
    if key in _CACHE:
        return _CACHE[key], _CACHE['wpack']
    consts = build_host_consts()
    wnames = sorted(consts.keys())
    wpack = np.zeros((P, len(wnames) * P), np.float32)
    for i, n in enumerate(wnames):
        wpack[:, i * P:(i + 1) * P] = consts[n]

    nc = bacc.Bacc("TRN2", target_bir_lowering=False, debug=False,
                   num_devices=B)
    x_d = nc.dram_tensor("x", [H, W], DT.float16, kind="ExternalInput").ap()
    p_d = nc.dram_tensor("prediction", [H, W], DT.int8,
                         kind="ExternalInput").ap()
    w_d = nc.dram_tensor("wpack", list(wpack.shape), DT.float32,
                         kind="ExternalInput").ap()
    y_d = nc.dram_tensor("y", [H, W], DT.float16, kind="ExternalOutput").ap()
    with tile.TileContext(nc) as tc:
        with ExitStack() as ctx:
            build_kernel(ctx, tc, [y_d], [x_d, p_d, w_d], reps=reps)
    nc.compile()
    _CACHE[('nc', reps)] = nc
    _CACHE['wpack'] = wpack
    return nc, wpack


def _get_exec(reps=1):
    """Compile (once) the 8-core sharded executable; stage constants."""
    key = ('exec', reps)
    if key in _CACHE:
        return _CACHE[key]
    import jax
    from jax.sharding import Mesh, PartitionSpec, NamedSharding
    from jax.experimental.shard_map import shard_map
    from concourse import bass2jax

    bass2jax.install_neuronx_cc_hook()
    nc, wpack = _build_program(reps)

    partition_name = (nc.partition_id_tensor.name
                      if nc.partition_id_tensor else None)
    in_names, out_names, out_avals = [], [], []
    for alloc in nc.m.functions[0].allocations:
        if not isinstance(alloc, mybir.MemoryLocationSet):
            continue
        name = alloc.memorylocations[0].name
        if alloc.kind == "ExternalInput":
            if name != partition_name:
                in_names.append(name)
        elif alloc.kind == "ExternalOutput":
            out_names.append(name)
            out_avals.append(jax.core.ShapedArray(
                tuple(alloc.tensor_shape), mybir.dt.np(alloc.dtype)))
    n_params = len(in_names)
    n_outs = len(out_names)

    devices = jax.devices()[:B]
    mesh = Mesh(np.asarray(devices), ("core",))
    shard = NamedSharding(mesh, PartitionSpec("core"))
    assert in_names == ['x', 'prediction', 'wpack'], in_names
    base_shapes = [
        jax.ShapeDtypeStruct((B * H, W), np.float16, sharding=shard),
        jax.ShapeDtypeStruct((B * H, W), np.int8, sharding=shard),
        jax.ShapeDtypeStruct((B * wpack.shape[0], wpack.shape[1]), np.float32,
                             sharding=shard),
    ]
    y_shape = jax.ShapeDtypeStruct((B * H, W), np.float16, sharding=shard)

    def make_compile_fn(with_y):
        # the kernel writes every y element, so the zero-filled y input
        # operand (run_bass_via_pjrt's donation scheme) is droppable if the
        # lowering accepts an output with no matching input operand
        all_names = list(in_names) + (list(out_names) if with_y else [])
        if partition_name is not None:
            all_names.append(partition_name)

        def _body(*args):
            operands = list(args)
            if partition_name is not None:
                operands.append(bass2jax.partition_id_tensor())
            outs = bass2jax._bass_exec_p.bind(
                *operands, out_avals=tuple(out_avals),
                in_names=tuple(all_names), out_names=tuple(out_names),
                lowering_input_output_aliases=(),
                sim_require_finite=True, sim_require_nnan=True, nc=nc)
            return tuple(outs)

        nin = n_params + (n_outs if with_y else 0)
        arg_shapes = base_shapes + ([y_shape] * n_outs if with_y else [])

        def compile_fn():
            jf = jax.jit(shard_map(
                _body, mesh=mesh,
                in_specs=(PartitionSpec("core"),) * nin,
                out_specs=(PartitionSpec("core"),) * n_outs,
                check_rep=False), keep_unused=True)
            return jf.lower(*arg_shapes).compile()
        return compile_fn

    with_y = False
    try:
        compiled = bass2jax.fast_dispatch_compile(make_compile_fn(False))
    except Exception:
        with_y = True
        try:
            compiled = bass2jax.fast_dispatch_compile(make_compile_fn(True))
        except Exception:
            compiled = make_compile_fn(True)()

    wd = jax.device_put(np.concatenate([wpack] * B, axis=0), shard)
    extra = (wd,)
    zd = None
    if with_y:
        zd = jax.device_put(np.zeros((B * H, W), np.float16), shard)
        extra = (wd, zd)
    jax.block_until_ready(extra)

    st = {'compiled': compiled, 'shard': shard, 'wd': wd, 'zd': zd,
          'extra': extra, 'with_y': with_y, 'nc': nc, 'wpack': wpack}
    _CACHE[('exec', reps)] = st
    return st


def _stage_inputs(x, prediction):
    """Host-compress + device_put with the executable's sharding."""
    import jax
    st = _get_exec()
    xs = np.ascontiguousarray(x.reshape(B * H, W)).astype(np.float16)
    ps = prediction.reshape(B * H, W).astype(np.int8)
    xd = jax.device_put(xs, st['shard'])
    pd = jax.device_put(np.ascontiguousarray(ps), st['shard'])
    return xd, pd


def kernel(x: np.ndarray, prediction: np.ndarray) -> np.ndarray:
    st = _get_exec()
    xd, pd = _stage_inputs(x, prediction)
    out = st['compiled'](xd, pd, *st['extra'])
    y = np.asarray(out[0]).astype(np.float32).reshape(B, 1, H, W)
    return y


if __name__ == "__main__":
    xs = np.random.randn(B, 1, H, W).astype(np.float32)
    ps = np.random.randint(0, 19, size=(B, 1, H, W)).astype(np.int32)
    print(kernel(xs, ps).shape)
